# revision 17
# baseline (speedup 1.0000x reference)
"""Causal self-attention (B=4, T=2048, D=1024, H=16) on 8 trn2 NeuronCores.

Sharding: batch (4-way) x head-half (2-way tensor parallel) => 8 cores,
one uniform SPMD program (per-core differences are pure data: which batch's
x, which half of the QKV columns / proj columns each core receives).

Per core (batch b, head-half hh, 8 local heads), all matmul operands bf16
(fp32 PSUM accumulation):
  1. QKV: q^T/k^T computed in [qkv_col, token] layout (lhsT = W chunk,
     rhs = x^T chunk); v computed in [token, vcol] layout.
  2. Attention per head pair, per 512-wide query tile, streaming 128-wide
     key blocks (block-causal; fully-masked key blocks are skipped):
       S^T[k,q]  = matmul(lhsT=k^T chunk, rhs=q^T tile)      (PSUM f32)
       both heads of a partition group run in disjoint PE row quadrants
       and land in one [128, 2, 512] PSUM tile, so a single ScalarE
       ACTIVATE computes exp for the pair (amortizes the ~175 ns fixed
       ACT cost, the ScalarE stream is the attention-phase floor).
       y_ext^T  += matmul(lhsT=v_ext block, rhs=P^T): v_ext carries a ones
                   column, so row HD of the accumulator is the softmax
                   denominator l -- no extra reduction pass.
  3. The whole kernel is emitted as one software pipeline over query
     tiles s: attention(s) first (highest priority), then norm+AllGather
     ship(s), then proj(s-1), then QKV(s+1). The Tile list scheduler
     back-fills PE idle slots (waiting on exp) with the ready QKV / proj
     matmuls, which keeps the PE HAM clock-gate warm (2.4 GHz) instead
     of oscillating against the ScalarE exp stream.
  4. Pairwise AllGather of y^T (bf16) between the two cores sharing a
     batch; proj is column-sharded => the host only concatenates.
"""

import os
import sys
from dataclasses import dataclass

import ml_dtypes
import numpy as np

sys.path.insert(0, "/opt/trn_rl_repo")

import concourse.mybir as mybir  # noqa: E402
import concourse.tile as tile  # noqa: E402
from concourse import bacc  # noqa: E402
from concourse.bass import ds, ts  # noqa: E402

P = 128
F32 = mybir.dt.float32
BF16 = mybir.dt.bfloat16
AF = mybir.ActivationFunctionType
ALU = mybir.AluOpType
BF16NP = ml_dtypes.bfloat16


@dataclass(frozen=True)
class Cfg:
    T: int = 2048          # sequence length
    D: int = 1024          # model dim (QKV contraction dim)
    H_LOC: int = 8         # heads per core
    HD: int = 64           # head dim
    TT: int = 512          # token tile width in the QKV phase
    QT: int = 512          # query tile width in the attention phase
    n_groups: int = 2      # cores sharing a batch (pairwise AllGather)
    scale: float = 64 ** -0.5

    @property
    def DH(self):          # local head dims (y^T rows contributed per core)
        return self.H_LOC * self.HD

    @property
    def GDH(self):         # proj contraction dim (= model dim)
        return self.n_groups * self.DH

    @property
    def DCH(self):
        return self.D // P

    @property
    def NHP(self):         # 128-partition groups of local head dims
        return self.DH // P

    @property
    def HPG(self):         # heads per 128-partition group
        return P // self.HD

    @property
    def NTT(self):
        return self.T // self.TT

    @property
    def NQT(self):
        return self.T // self.QT

    @property
    def CB(self):          # 128-wide column blocks of the local q/k cols
        return self.DH // P


FULL = Cfg()


def build_nc(c: Cfg, n_cores: int = 8, with_bias: bool = True):
    """Build the (uniform SPMD) Bass program for one core."""
    assert c.T % c.TT == 0 and c.T % c.QT == 0 and c.QT % P == 0
    assert c.D % P == 0 and c.DH % P == 0 and c.TT % P == 0
    use_cc = c.n_groups > 1

    nc = bacc.Bacc(
        "TRN2", target_bir_lowering=False, debug=False, num_devices=n_cores
    )
    # x and the weights arrive pre-arranged from the host (partition-major,
    # contiguous per partition) so every load is a full-line DMA
    xT = nc.dram_tensor("xT", [P, c.NTT, c.DCH, c.TT], BF16,
                        kind="ExternalInput").ap()
    wq = nc.dram_tensor("wq", [P, c.DCH, c.DH], BF16,
                        kind="ExternalInput").ap()
    wk = nc.dram_tensor("wk", [P, c.DCH, c.DH], BF16,
                        kind="ExternalInput").ap()
    wv = nc.dram_tensor("wv", [P, c.DCH, c.DH], BF16,
                        kind="ExternalInput").ap()
    bq = nc.dram_tensor("bq", [c.DH], F32, kind="ExternalInput").ap()
    bk = nc.dram_tensor("bk", [c.DH], F32, kind="ExternalInput").ap()
    bv = nc.dram_tensor("bv", [1, c.DH], BF16, kind="ExternalInput").ap()
    wp = nc.dram_tensor("wp", [P, c.GDH // P, c.DH], BF16,
                        kind="ExternalInput").ap()
    bp = nc.dram_tensor("bp", [1, c.DH], BF16, kind="ExternalInput").ap()
    oc = max(P, (c.T // P) * c.H_LOC)
    onesin = nc.dram_tensor("onesin", [P, oc], BF16, kind="ExternalInput").ap()
    esel = nc.dram_tensor("esel", [c.H_LOC, c.NHP * P], BF16,
                          kind="ExternalInput").ap()
    out = nc.dram_tensor("out", [c.T, c.DH], F32, kind="ExternalOutput").ap()

    groups = [[g * c.n_groups + i for i in range(c.n_groups)]
              for g in range(max(1, n_cores // c.n_groups))]

    with tile.TileContext(nc) as tc:
        with (
            tc.tile_pool(name="const", bufs=1) as cst,
            tc.tile_pool(name="kv", bufs=1) as kv,
            tc.tile_pool(name="wqkv", bufs=1) as wqk,
            tc.tile_pool(name="wproj", bufs=1) as wpp,
            tc.tile_pool(name="xt", bufs=2) as xtp,
            tc.tile_pool(name="pt", bufs=4) as ptp,
            tc.tile_pool(name="yt", bufs=2) as ytp,
            tc.tile_pool(name="yu", bufs=2) as yup,
            tc.tile_pool(name="lr", bufs=2) as lrp,
            tc.tile_pool(name="yag", bufs=2) as yagp,
            tc.tile_pool(name="osb", bufs=2) as osbp,
            tc.tile_pool(name="ps_mm", bufs=2, space="PSUM") as ps_mm,
            tc.tile_pool(name="ps_s", bufs=2, space="PSUM") as ps_s,
            tc.tile_pool(name="ps_y", bufs=2, space="PSUM") as ps_y,
            tc.tile_pool(name="dram", bufs=2, space="DRAM") as drp,
        ):
            # ---- constants ----
            ones_row = cst.tile([1, P], BF16)
            nc.gpsimd.dma_start(ones_row, onesin[0:1, 0:P])
            bq_sb = cst.tile([P, c.CB], F32)
            nc.scalar.dma_start(bq_sb, bq.rearrange("(cb p) -> p cb", p=P))
            bk_sb = cst.tile([P, c.CB], F32)
            nc.scalar.dma_start(bk_sb, bk.rearrange("(cb p) -> p cb", p=P))
            bv_row = cst.tile([1, c.DH], BF16)
            nc.scalar.dma_start(bv_row, bv)
            bp_row = cst.tile([1, c.DH], BF16)
            nc.scalar.dma_start(bp_row, bp)
            esel_sb = cst.tile([c.H_LOC, c.NHP * P], BF16)
            nc.gpsimd.dma_start(esel_sb, esel)
            # pre-load the exp spline table so the ~2.7us ACT_TABLE_LOAD
            # overlaps the QKV prologue instead of the first S^T block
            warm = cst.tile([1, 8], BF16)
            nc.scalar.activation(warm, ones_row[0:1, 0:8], AF.Exp)
            # causal triangle mask (keep j >= k) for the diagonal key block,
            # replicated per head of a partition group so one DVE multiply
            # masks the whole [128, HPG, 128] slice
            mask2 = cst.tile([P, c.HPG, P], BF16)
            nc.vector.memset(mask2, 1.0)
            for i in range(c.HPG):
                nc.gpsimd.affine_select(
                    mask2[:, i, :], mask2[:, i, :],
                    compare_op=ALU.is_ge, fill=0.0, base=0,
                    pattern=[[1, P]], channel_multiplier=-1,
                )

            # ---- persistent K^T / Q^T / V(+ones) ----
            kT = kv.tile([P, c.NHP, c.T], BF16)
            qT = kv.tile([P, c.NHP, c.T], BF16)
            v = kv.tile([P, c.T // P, c.H_LOC, c.HD + 1], BF16)
            nc.vector.memset(v[:, :, :, c.HD:c.HD + 1], 1.0)

            wq_sb = wqk.tile([P, c.DCH, c.DH], BF16)
            wk_sb = wqk.tile([P, c.DCH, c.DH], BF16)
            wv_sb = wqk.tile([P, c.DCH, c.DH], BF16)
            wp_sb = wpp.tile([P, c.GDH // P, c.DH], BF16)
            # per-dc chunks, interleaved: the first K/Q accumulation
            # chains become runnable after ~2 chunks instead of after the
            # whole 4 MB of weights+x (the prologue is DMA-bound)
            x0 = xtp.tile([P, c.DCH, c.TT], BF16, name="x0")
            for dc in range(c.DCH):
                nc.sync.dma_start(x0[:, dc, :], xT[:, 0, dc, :])
                nc.gpsimd.dma_start(wk_sb[:, dc, :], wk[:, dc, :])
                nc.scalar.dma_start(wq_sb[:, dc, :], wq[:, dc, :])
                nc.gpsimd.dma_start(wv_sb[:, dc, :], wv[:, dc, :])

            def qkv_kq(xt, tt, cb):
                for dst, w_sb, b_sb in (
                    (kT, wk_sb, bk_sb),
                    (qT, wq_sb, bq_sb),
                ):
                    pst = ps_mm.tile([P, max(c.TT, c.DH)], F32,
                                     tag="mm", name="pst")[:, :c.TT]
                    for dc in range(c.DCH):
                        nc.tensor.matmul(
                            pst,
                            w_sb[:, dc, ts(cb, P)],
                            xt[:, dc, :],
                            start=(dc == 0),
                            stop=(dc == c.DCH - 1),
                        )
                    nc.vector.tensor_tensor(
                        dst[:, cb, ts(tt, c.TT)], pst,
                        b_sb[:, cb:cb + 1].to_broadcast((P, c.TT)),
                        ALU.add,
                    )

            def qkv_v(xt, tt, tb):
                gtb = tt * (c.TT // P) + tb
                psv = ps_mm.tile([P, max(c.TT, c.DH)], F32,
                                 tag="mm", name="psv")[:, :c.DH]
                for dc in range(c.DCH):
                    nc.tensor.matmul(
                        psv,
                        xt[:, dc, ts(tb, P)],
                        wv_sb[:, dc, :],
                        start=(dc == 0),
                        stop=(not with_bias and dc == c.DCH - 1),
                    )
                if with_bias:
                    nc.tensor.matmul(
                        psv, ones_row[0:1, 0:P], bv_row,
                        start=False, stop=True,
                    )
                nc.vector.tensor_copy(
                    v[:, gtb, :, 0:c.HD],
                    psv.rearrange("p (h d) -> p h d", d=c.HD),
                )

            def qkv_tile(tt, xt=None):
                if xt is None:
                    xt = xtp.tile([P, c.DCH, c.TT], BF16)
                    nc.sync.dma_start(xt, xT[:, tt, :, :])
                # K/Q col-block 0 and all of V first: the first attention
                # partition group of the NEXT query tile becomes runnable
                # as early as possible (matters most for the prologue tile)
                qkv_kq(xt, tt, 0)
                for tb in range(c.TT // P):
                    qkv_v(xt, tt, tb)
                for cb in range(1, c.CB):
                    qkv_kq(xt, tt, cb)

            def attention(q0, qw, fillers=None):
                nkb = (q0 + qw) // P
                yu_q = yup.tile([P, c.NHP, c.QT], F32, name="yu_q")[:, :, :qw]
                l_all = lrp.tile([c.H_LOC, c.QT], F32, tag="lall",
                                  name="l_all")[:, :qw]
                for hp in range(c.NHP):
                    # the HPG heads sharing this partition group run their
                    # S^T matmuls in disjoint PE row quadrants
                    # (tile_position auto-derived from base_partition) into
                    # one 2-bank PSUM tile => a single ACTIVATE per block.
                    psys = [ps_y.tile([c.HD + 1, c.QT], F32, tag="psy",
                                      name=f"psy{hs}")[:, :qw]
                            for hs in range(c.HPG)]
                    for kb in range(nkb):
                        off = max(0, kb * P - q0)
                        diag = kb * P - q0 >= 0
                        w = qw - off
                        pss = ps_s.tile([P, c.HPG, c.QT], F32, tag="pss",
                                        name="pss")[:, :, :qw]
                        for hs in range(c.HPG):
                            pb = hs * c.HD
                            nc.tensor.matmul(
                                pss[:, hs, off:],
                                kT[pb:pb + c.HD, hp, ts(kb, P)],
                                qT[pb:pb + c.HD, hp, ds(q0 + off, w)],
                                start=True, stop=True,
                            )
                        pt = ptp.tile([P, c.HPG, c.QT], BF16, tag="pt",
                                      name="pt")[:, :, :qw]
                        nc.scalar.activation(
                            pt[:, :, off:], pss[:, :, off:],
                            AF.Exp, scale=c.scale)
                        if diag:
                            # triangle mask on the 128 cols at the diagonal
                            nc.vector.tensor_tensor(
                                pt[:, :, off:off + P],
                                pt[:, :, off:off + P],
                                mask2, ALU.mult,
                            )
                        for hs in range(c.HPG):
                            nc.tensor.matmul(
                                psys[hs][:, off:],
                                v[:, kb, hp * c.HPG + hs, :],
                                pt[:, hs, off:],
                                start=(kb == 0),
                                stop=(kb == nkb - 1),
                            )
                    # stage the denominators first (they head the serial
                    # reciprocal -> norm -> ship chain), then y^T
                    l_sbs = []
                    for hs in range(c.HPG):
                        l_sb = lrp.tile([1, c.QT], F32, tag="lsb",
                                        name="l_sb")[:, :qw]
                        nc.vector.tensor_copy(
                            l_sb, psys[hs][c.HD:c.HD + 1, :])
                        nc.gpsimd.dma_start(
                            l_all[hp * c.HPG + hs:hp * c.HPG + hs + 1, :],
                            l_sb)
                        l_sbs.append(l_sb)
                    for hs in range(c.HPG):
                        pb = hs * c.HD
                        nc.vector.tensor_copy(
                            yu_q[pb:pb + c.HD, hp, :], psys[hs][0:c.HD, :])
                    # hp-boundary fillers: QKV chains of LATER tiles, spread
                    # into this (scalar-bound) region. All writes still
                    # precede their readers in emission order.
                    if fillers:
                        for th in fillers[hp]:
                            th()
                return yu_q, l_all

            def norm(yu_q, l_all, qw):
                # batched softmax normalization for all 8 heads; 1/l only
                # needs bf16 accuracy downstream, so the ~18-bit single-op
                # DVE approximation replaces the slow exact reciprocal
                r32 = lrp.tile([c.H_LOC, c.QT], F32, tag="r32",
                               name="r32")[:, :qw]
                nc.vector.reciprocal_approx_fast(r32, l_all)
                r_all = lrp.tile([c.H_LOC, c.QT], BF16, tag="rall",
                                 name="r_all")[:, :qw]
                nc.vector.tensor_copy(r_all, r32)
                yt_q = ytp.tile([P, c.NHP, c.QT], BF16, name="yt_q")[:, :, :qw]
                for hp in range(c.NHP):
                    psr = ps_mm.tile([P, max(c.TT, c.DH)], F32,
                                     tag="mm", name="psr")[:, :qw]
                    nc.tensor.matmul(
                        psr, esel_sb[:, ts(hp, P)], r_all,
                        start=True, stop=True,
                    )
                    nc.vector.tensor_tensor(
                        yt_q[:, hp, :], yu_q[:, hp, :], psr, ALU.mult,
                    )
                return yt_q

            def ship(q0, qw, yt_q, pieces=1):
                # ship y^T; pairwise AllGather along the dims axis. The
                # final step ships in two half-width pieces: small
                # AllGathers have much lower latency and the first piece's
                # proj overlaps the second piece's gather.
                shipped = []
                pw = qw // pieces
                for pc in range(pieces):
                    y_loc = drp.tile([c.DH, pw], BF16,
                                     tag=f"yloc{pw}", name="y_loc")
                    nc.sync.dma_start(
                        y_loc.rearrange("(hp p) t -> p hp t", p=P),
                        yt_q[:, :, ds(pc * pw, pw)],
                    )
                    if use_cc:
                        y_ag = drp.tile([c.GDH, pw], BF16,
                                        tag=f"ygat{pw}", name="y_ag")
                        nc.gpsimd.collective_compute(
                            "AllGather", ALU.bypass,
                            replica_groups=groups,
                            ins=[y_loc.opt()], outs=[y_ag.opt()],
                        )
                    else:
                        y_ag = y_loc
                    shipped.append((y_ag, pw, q0 + pc * pw))
                return shipped

            def proj(y_ag, hw_, tok0):
                # column-sharded proj on the gathered y => the host only
                # concatenates, no reduction anywhere
                yag_sb = yagp.tile([P, c.GDH // P, c.QT], BF16,
                                   name="yag_sb")[:, :, :hw_]
                nc.sync.dma_start(
                    yag_sb,
                    y_ag.rearrange("(ch p) t -> p ch t", p=P),
                )
                for tb in range(hw_ // P):
                    pso = ps_mm.tile([P, max(c.TT, c.DH)], F32,
                                     tag="mm", name="pso")[:, :c.DH]
                    for c2 in range(c.GDH // P):
                        nc.tensor.matmul(
                            pso,
                            yag_sb[:, c2, ts(tb, P)],
                            wp_sb[:, c2, :],
                            start=(c2 == 0),
                            stop=(not with_bias
                                  and c2 == c.GDH // P - 1),
                        )
                    if with_bias:
                        nc.tensor.matmul(
                            pso, ones_row[0:1, 0:P], bp_row,
                            start=False, stop=True,
                        )
                    osb = osbp.tile([P, c.DH], F32)
                    nc.vector.tensor_copy(osb, pso)
                    nc.gpsimd.dma_start(out[ds(tok0 + tb * P, P), :], osb)

            # ---- software pipeline over query-tile steps ----
            # The last tile is split in two half-width attention passes so
            # its drain/AllGather hides under the second half's compute.
            # Emission order per step: attention -> next QKV tile ->
            # proj(prev) -> norm -> ship. QKV/proj sit between attention
            # and norm in mm-pool allocation order, so their PSUM slots
            # recycle long-released tiles and the Tile list scheduler can
            # back-fill PE idle slots (ScalarE exp waits) with them.
            steps = [(qt * c.QT, c.QT) for qt in range(c.NQT)]
            if c.QT // 2 >= P:
                q0l = steps.pop()[0]
                h = c.QT // 2
                steps += [(q0l, h), (q0l + h, h)]
            balanced = c.NQT == 4 and c.NTT == 4 and len(steps) == 5

            def xt_dma(tt):
                xt = xtp.tile([P, c.DCH, c.TT], BF16, name="xt")
                nc.sync.dma_start(xt, xT[:, tt, :, :])
                return xt

            if balanced:
                # Spread QKV tiles 2 and 3 over the scalar-bound attention
                # regions at hp boundaries: steps 1-2 are PE-bound while
                # the late small-step regions idle the PE, so chains of the
                # NEXT tiles migrate late (each chain still lands before
                # the first attention unit that reads it).
                x2, x3 = [None], [None]

                def kq(xh, tt, cb):
                    return lambda: qkv_kq(xh[0], tt, cb)

                def vv(xh, tt, tb):
                    return lambda: qkv_v(xh[0], tt, tb)

                fill = {
                    1: [[kq(x2, 2, 0)], [kq(x2, 2, 1)],
                        [vv(x2, 2, 0), vv(x2, 2, 1)],
                        [vv(x2, 2, 2), vv(x2, 2, 3)]],
                    2: [[kq(x2, 2, 2)], [kq(x2, 2, 3)],
                        [kq(x3, 3, 0)],
                        [vv(x3, 3, 0), vv(x3, 3, 1)]],
                    3: [[kq(x3, 3, 1), vv(x3, 3, 2)],
                        [kq(x3, 3, 2), vv(x3, 3, 3)],
                        [kq(x3, 3, 3)], []],
                }
            else:
                # attention step s uses K/V tiles <= s, so tile s+1 is
                # emitted during step s
                fill = {}
                qkv_after = {s: s + 1 for s in range(c.NQT - 1)}
            qkv_tile(0, xt=x0)
            pending = []
            for s, (q0, qw) in enumerate(steps):
                last = s == len(steps) - 1
                if balanced and s == 1:
                    x2[0] = xt_dma(2)
                if balanced and s == 2:
                    x3[0] = xt_dma(3)
                yu_q, l_all = attention(q0, qw, fill.get(s))
                if balanced:
                    if s == 0:
                        qkv_tile(1)
                else:
                    nt = qkv_after.get(s)
                    if nt is not None and nt < c.NTT:
                        qkv_tile(nt)
                if s == 0:
                    # wp is first needed by proj during step 1; deferring
                    # its DMA keeps startup bandwidth for x and w_qkv
                    nc.gpsimd.dma_start(wp_sb, wp)
                for item in pending:
                    proj(*item)
                # norm/ship head the per-step serial chain into the
                # AllGather: boost them above the qkv/proj filler so the
                # collective is issued as soon as the drain lands
                with tc.high_priority():
                    yt_q = norm(yu_q, l_all, qw)
                    pending = ship(q0, qw, yt_q,
                                   pieces=2 if (last and qw // 2 >= P)
                                   else 1)
            for item in pending:
                proj(*item)

    nc.compile()
    return nc


def arrange_x(c: Cfg, xT):
    """[D, T] -> [P, NTT, DCH, TT]: partition-major, contiguous lines."""
    return np.ascontiguousarray(
        xT.reshape(c.DCH, P, c.NTT, c.TT).transpose(1, 2, 0, 3))


def arrange_w(c: Cfg, w):
    """[nch*P, n] -> [P, nch, n]: partition-major, contiguous lines."""
    nch = w.shape[0] // P
    return np.ascontiguousarray(
        w.reshape(nch, P, w.shape[1]).transpose(1, 0, 2))


def shard_inputs(c: Cfg, x, w_qkv, b_qkv, w_proj, b_proj, n_cores=8):
    """Full fp32 inputs -> per-core input maps (host-side marshalling).

    Matmul operands are cast to bf16 on the host; q/k biases stay fp32
    (applied via ScalarE's per-partition bias port on the f32 PSUM)."""
    D, DH = c.D, c.DH
    oc = max(128, (c.T // 128) * c.H_LOC)
    ones = np.ones((128, oc), BF16NP)
    esel = np.zeros((c.H_LOC, c.NHP * 128), BF16NP)
    for h in range(c.H_LOC):
        hp, sub = h // c.HPG, h % c.HPG
        esel[h, hp * 128 + sub * c.HD: hp * 128 + (sub + 1) * c.HD] = 1
    maps = []
    for core in range(n_cores):
        b, hh = core // c.n_groups, core % c.n_groups
        sl = slice(hh * DH, (hh + 1) * DH)
        maps.append({
            "xT": arrange_x(c, np.ascontiguousarray(x[b].T)).astype(BF16NP),
            "wq": arrange_w(
                c, w_qkv[:, 0 * D:1 * D][:, sl]).astype(BF16NP),
            "wk": arrange_w(
                c, w_qkv[:, 1 * D:2 * D][:, sl]).astype(BF16NP),
            "wv": arrange_w(
                c, w_qkv[:, 2 * D:3 * D][:, sl]).astype(BF16NP),
            "bq": np.ascontiguousarray(
                b_qkv[0 * D:1 * D][sl], dtype=np.float32),
            "bk": np.ascontiguousarray(
                b_qkv[1 * D:2 * D][sl], dtype=np.float32),
            "bv": np.ascontiguousarray(
                b_qkv[2 * D:3 * D][sl]).reshape(1, DH).astype(BF16NP),
            "wp": arrange_w(c, w_proj[:, sl]).astype(BF16NP),
            "bp": np.ascontiguousarray(
                b_proj[sl]).reshape(1, DH).astype(BF16NP),
            "onesin": ones,
            "esel": esel,
        })
    return maps


def gather_outputs(c: Cfg, results, n_cores=8):
    B = n_cores // c.n_groups
    out = np.empty((B, c.T, c.GDH), dtype=np.float32)
    for core in range(n_cores):
        b, hh = core // c.n_groups, core % c.n_groups
        out[b][:, hh * c.DH:(hh + 1) * c.DH] = results[core]["out"]
    return out


_NC_CACHE: dict = {}


def kernel(**inputs) -> np.ndarray:
    from concourse.bass_utils import run_bass_kernel_spmd

    c = FULL
    n_cores = 8
    wb = bool(np.any(inputs["b_qkv"]) or np.any(inputs["b_proj"]))
    key = (c, n_cores, wb)
    if key not in _NC_CACHE:
        _NC_CACHE[key] = build_nc(c, n_cores, with_bias=wb)
    nc = _NC_CACHE[key]
    in_maps = shard_inputs(
        c, inputs["x"], inputs["w_qkv"], inputs["b_qkv"],
        inputs["w_proj"], inputs["b_proj"], n_cores,
    )
    res = run_bass_kernel_spmd(
        nc, in_maps, core_ids=list(range(n_cores)),
        trace=bool(int(os.environ.get("KERNEL_TRACE", "0"))),
    )
    kernel.last_results = res
    return gather_outputs(c, res.results, n_cores)


# revision 19
# speedup vs baseline: 1.2002x; 1.2002x over previous
"""Causal self-attention (B=4, T=2048, D=1024, H=16) on 8 trn2 NeuronCores.

Sharding: batch (4-way) x head-half (2-way tensor parallel) => 8 cores,
one uniform SPMD program (per-core differences are pure data: which batch's
x, which half of the QKV columns / proj columns each core receives).

Per core (batch b, head-half hh, 8 local heads), all matmul operands bf16
(fp32 PSUM accumulation):
  1. QKV: q^T/k^T computed in [qkv_col, token] layout (lhsT = W chunk,
     rhs = x^T chunk); v computed in [token, vcol] layout.
  2. Attention per head pair, per 512-wide query tile, streaming 128-wide
     key blocks (block-causal; fully-masked key blocks are skipped):
       S^T[k,q]  = matmul(lhsT=k^T chunk, rhs=q^T tile)      (PSUM f32)
       both heads of a partition group run in disjoint PE row quadrants
       and land in one [128, 2, 512] PSUM tile, so a single ScalarE
       ACTIVATE computes exp for the pair (amortizes the ~175 ns fixed
       ACT cost, the ScalarE stream is the attention-phase floor).
       y_ext^T  += matmul(lhsT=v_ext block, rhs=P^T): v_ext carries a ones
                   column, so row HD of the accumulator is the softmax
                   denominator l -- no extra reduction pass.
  3. The whole kernel is emitted as one software pipeline over query
     tiles s: attention(s) first (highest priority), then norm+AllGather
     ship(s), then proj(s-1), then QKV(s+1). The Tile list scheduler
     back-fills PE idle slots (waiting on exp) with the ready QKV / proj
     matmuls, which keeps the PE HAM clock-gate warm (2.4 GHz) instead
     of oscillating against the ScalarE exp stream.
  4. Pairwise AllGather of y^T (bf16) between the two cores sharing a
     batch; proj is column-sharded => the host only concatenates.
"""

import os
import sys
from dataclasses import dataclass

import ml_dtypes
import numpy as np

sys.path.insert(0, "/opt/trn_rl_repo")

import concourse.mybir as mybir  # noqa: E402
import concourse.tile as tile  # noqa: E402
from concourse import bacc  # noqa: E402
from concourse.bass import ds, ts  # noqa: E402

P = 128
F32 = mybir.dt.float32
BF16 = mybir.dt.bfloat16
AF = mybir.ActivationFunctionType
ALU = mybir.AluOpType
BF16NP = ml_dtypes.bfloat16


@dataclass(frozen=True)
class Cfg:
    T: int = 2048          # sequence length
    D: int = 1024          # model dim (QKV contraction dim)
    H_LOC: int = 8         # heads per core
    HD: int = 64           # head dim
    TT: int = 512          # token tile width in the QKV phase
    QT: int = 512          # query tile width in the attention phase
    n_groups: int = 2      # cores sharing a batch (pairwise AllGather)
    scale: float = 64 ** -0.5

    @property
    def DH(self):          # local head dims (y^T rows contributed per core)
        return self.H_LOC * self.HD

    @property
    def GDH(self):         # proj contraction dim (= model dim)
        return self.n_groups * self.DH

    @property
    def DCH(self):
        return self.D // P

    @property
    def NHP(self):         # 128-partition groups of local head dims
        return self.DH // P

    @property
    def HPG(self):         # heads per 128-partition group
        return P // self.HD

    @property
    def NTT(self):
        return self.T // self.TT

    @property
    def NQT(self):
        return self.T // self.QT

    @property
    def CB(self):          # 128-wide column blocks of the local q/k cols
        return self.DH // P


FULL = Cfg()


def build_nc(c: Cfg, n_cores: int = 8, with_bias: bool = True):
    """Build the (uniform SPMD) Bass program for one core."""
    assert c.T % c.TT == 0 and c.T % c.QT == 0 and c.QT % P == 0
    assert c.D % P == 0 and c.DH % P == 0 and c.TT % P == 0
    use_cc = c.n_groups > 1

    nc = bacc.Bacc(
        "TRN2", target_bir_lowering=False, debug=False, num_devices=n_cores
    )
    # x and the weights arrive pre-arranged from the host (partition-major,
    # contiguous per partition) so every load is a full-line DMA
    xT = nc.dram_tensor("xT", [P, c.NTT, c.DCH, c.TT], BF16,
                        kind="ExternalInput").ap()
    wq = nc.dram_tensor("wq", [P, c.DCH, c.DH], BF16,
                        kind="ExternalInput").ap()
    wk = nc.dram_tensor("wk", [P, c.DCH, c.DH], BF16,
                        kind="ExternalInput").ap()
    wv = nc.dram_tensor("wv", [P, c.DCH, c.DH], BF16,
                        kind="ExternalInput").ap()
    bq = nc.dram_tensor("bq", [c.DH], F32, kind="ExternalInput").ap()
    bk = nc.dram_tensor("bk", [c.DH], F32, kind="ExternalInput").ap()
    bv = nc.dram_tensor("bv", [1, c.DH], BF16, kind="ExternalInput").ap()
    wp = nc.dram_tensor("wp", [P, c.GDH // P, c.DH], BF16,
                        kind="ExternalInput").ap()
    bp = nc.dram_tensor("bp", [1, c.DH], BF16, kind="ExternalInput").ap()
    oc = max(P, (c.T // P) * c.H_LOC)
    onesin = nc.dram_tensor("onesin", [P, oc], BF16, kind="ExternalInput").ap()
    esel = nc.dram_tensor("esel", [c.H_LOC, c.NHP * P], BF16,
                          kind="ExternalInput").ap()
    out = nc.dram_tensor("out", [c.T, c.DH], F32, kind="ExternalOutput").ap()

    groups = [[g * c.n_groups + i for i in range(c.n_groups)]
              for g in range(max(1, n_cores // c.n_groups))]

    with tile.TileContext(nc) as tc:
        with (
            tc.tile_pool(name="const", bufs=1) as cst,
            tc.tile_pool(name="kv", bufs=1) as kv,
            tc.tile_pool(name="wqkv", bufs=1) as wqk,
            tc.tile_pool(name="wproj", bufs=1) as wpp,
            tc.tile_pool(name="xt", bufs=3) as xtp,
            tc.tile_pool(name="pt", bufs=4) as ptp,
            tc.tile_pool(name="yt", bufs=2) as ytp,
            tc.tile_pool(name="yu", bufs=2) as yup,
            tc.tile_pool(name="lr", bufs=2) as lrp,
            tc.tile_pool(name="yag", bufs=2) as yagp,
            tc.tile_pool(name="osb", bufs=2) as osbp,
            tc.tile_pool(name="ps_mm", bufs=2, space="PSUM") as ps_mm,
            tc.tile_pool(name="ps_s", bufs=2, space="PSUM") as ps_s,
            tc.tile_pool(name="ps_y", bufs=2, space="PSUM") as ps_y,
            tc.tile_pool(name="dram", bufs=2, space="DRAM") as drp,
        ):
            # ---- constants ----
            ones_row = cst.tile([1, P], BF16)
            nc.gpsimd.dma_start(ones_row, onesin[0:1, 0:P])
            bq_sb = cst.tile([P, c.CB], F32)
            nc.scalar.dma_start(bq_sb, bq.rearrange("(cb p) -> p cb", p=P))
            bk_sb = cst.tile([P, c.CB], F32)
            nc.scalar.dma_start(bk_sb, bk.rearrange("(cb p) -> p cb", p=P))
            bv_row = cst.tile([1, c.DH], BF16)
            nc.scalar.dma_start(bv_row, bv)
            bp_row = cst.tile([1, c.DH], BF16)
            nc.scalar.dma_start(bp_row, bp)
            esel_sb = cst.tile([c.H_LOC, c.NHP * P], BF16)
            nc.gpsimd.dma_start(esel_sb, esel)
            # pre-load the exp spline table so the ~2.7us ACT_TABLE_LOAD
            # overlaps the QKV prologue instead of the first S^T block
            warm = cst.tile([1, 8], BF16)
            nc.scalar.activation(warm, ones_row[0:1, 0:8], AF.Exp)
            # causal triangle mask (keep j >= k) for the diagonal key block,
            # replicated per head of a partition group so one DVE multiply
            # masks the whole [128, HPG, 128] slice
            mask2 = cst.tile([P, c.HPG, P], BF16)
            nc.vector.memset(mask2, 1.0)
            for i in range(c.HPG):
                nc.gpsimd.affine_select(
                    mask2[:, i, :], mask2[:, i, :],
                    compare_op=ALU.is_ge, fill=0.0, base=0,
                    pattern=[[1, P]], channel_multiplier=-1,
                )

            # ---- persistent K^T / Q^T / V(+ones) ----
            kT = kv.tile([P, c.NHP, c.T], BF16)
            qT = kv.tile([P, c.NHP, c.T], BF16)
            v = kv.tile([P, c.T // P, c.H_LOC, c.HD + 1], BF16)
            nc.vector.memset(v[:, :, :, c.HD:c.HD + 1], 1.0)

            wq_sb = wqk.tile([P, c.DCH, c.DH], BF16)
            wk_sb = wqk.tile([P, c.DCH, c.DH], BF16)
            wv_sb = wqk.tile([P, c.DCH, c.DH], BF16)
            wp_sb = wpp.tile([P, c.GDH // P, c.DH], BF16)
            # per-dc chunks, interleaved: the first K/Q accumulation
            # chains become runnable after ~2 chunks instead of after the
            # whole 4 MB of weights+x (the prologue is DMA-bound)
            x0 = xtp.tile([P, c.DCH, c.TT], BF16, name="x0")
            for dc in range(c.DCH):
                nc.sync.dma_start(x0[:, dc, :], xT[:, 0, dc, :])
                nc.gpsimd.dma_start(wk_sb[:, dc, :], wk[:, dc, :])
                nc.scalar.dma_start(wq_sb[:, dc, :], wq[:, dc, :])
                nc.gpsimd.dma_start(wv_sb[:, dc, :], wv[:, dc, :])

            def qkv_kq(xt, tt, cb):
                for dst, w_sb, b_sb in (
                    (kT, wk_sb, bk_sb),
                    (qT, wq_sb, bq_sb),
                ):
                    pst = ps_mm.tile([P, max(c.TT, c.DH)], F32,
                                     tag="mm", name="pst")[:, :c.TT]
                    for dc in range(c.DCH):
                        nc.tensor.matmul(
                            pst,
                            w_sb[:, dc, ts(cb, P)],
                            xt[:, dc, :],
                            start=(dc == 0),
                            stop=(dc == c.DCH - 1),
                        )
                    nc.vector.tensor_tensor(
                        dst[:, cb, ts(tt, c.TT)], pst,
                        b_sb[:, cb:cb + 1].to_broadcast((P, c.TT)),
                        ALU.add,
                    )

            def qkv_v(xt, tt, tb):
                gtb = tt * (c.TT // P) + tb
                psv = ps_mm.tile([P, max(c.TT, c.DH)], F32,
                                 tag="mm", name="psv")[:, :c.DH]
                for dc in range(c.DCH):
                    nc.tensor.matmul(
                        psv,
                        xt[:, dc, ts(tb, P)],
                        wv_sb[:, dc, :],
                        start=(dc == 0),
                        stop=(not with_bias and dc == c.DCH - 1),
                    )
                if with_bias:
                    nc.tensor.matmul(
                        psv, ones_row[0:1, 0:P], bv_row,
                        start=False, stop=True,
                    )
                nc.vector.tensor_copy(
                    v[:, gtb, :, 0:c.HD],
                    psv.rearrange("p (h d) -> p h d", d=c.HD),
                )

            def xt_dma(tt):
                # one dma_start entry is serviced by a single DMA engine
                # (~19 GB/s): split every big transfer into per-chunk
                # entries so they spread across the ring's 16 engines
                xt = xtp.tile([P, c.DCH, c.TT], BF16, name="xt")
                for dc in range(c.DCH):
                    nc.sync.dma_start(xt[:, dc, :], xT[:, tt, dc, :])
                return xt

            def qkv_tile(tt, xt=None):
                if xt is None:
                    xt = xt_dma(tt)
                # K/Q col-block 0 and all of V first: the first attention
                # partition group of the NEXT query tile becomes runnable
                # as early as possible (matters most for the prologue tile)
                qkv_kq(xt, tt, 0)
                for tb in range(c.TT // P):
                    qkv_v(xt, tt, tb)
                for cb in range(1, c.CB):
                    qkv_kq(xt, tt, cb)

            def attention(q0, qw, fillers=None):
                nkb = (q0 + qw) // P
                yu_q = yup.tile([P, c.NHP, c.QT], F32, name="yu_q")[:, :, :qw]
                l_all = lrp.tile([c.H_LOC, c.QT], F32, tag="lall",
                                  name="l_all")[:, :qw]
                for hp in range(c.NHP):
                    # the HPG heads sharing this partition group run their
                    # S^T matmuls in disjoint PE row quadrants
                    # (tile_position auto-derived from base_partition) into
                    # one 2-bank PSUM tile => a single ACTIVATE per block.
                    psys = [ps_y.tile([c.HD + 1, c.QT], F32, tag="psy",
                                      name=f"psy{hs}")[:, :qw]
                            for hs in range(c.HPG)]
                    for kb in range(nkb):
                        off = max(0, kb * P - q0)
                        diag = kb * P - q0 >= 0
                        w = qw - off
                        pss = ps_s.tile([P, c.HPG, c.QT], F32, tag="pss",
                                        name="pss")[:, :, :qw]
                        for hs in range(c.HPG):
                            pb = hs * c.HD
                            nc.tensor.matmul(
                                pss[:, hs, off:],
                                kT[pb:pb + c.HD, hp, ts(kb, P)],
                                qT[pb:pb + c.HD, hp, ds(q0 + off, w)],
                                start=True, stop=True,
                            )
                        pt = ptp.tile([P, c.HPG, c.QT], BF16, tag="pt",
                                      name="pt")[:, :, :qw]
                        nc.scalar.activation(
                            pt[:, :, off:], pss[:, :, off:],
                            AF.Exp, scale=c.scale)
                        if diag:
                            # triangle mask on the 128 cols at the diagonal
                            nc.vector.tensor_tensor(
                                pt[:, :, off:off + P],
                                pt[:, :, off:off + P],
                                mask2, ALU.mult,
                            )
                        for hs in range(c.HPG):
                            nc.tensor.matmul(
                                psys[hs][:, off:],
                                v[:, kb, hp * c.HPG + hs, :],
                                pt[:, hs, off:],
                                start=(kb == 0),
                                stop=(kb == nkb - 1),
                            )
                    # stage the denominators first (they head the serial
                    # reciprocal -> norm -> ship chain), then y^T
                    l_sbs = []
                    for hs in range(c.HPG):
                        l_sb = lrp.tile([1, c.QT], F32, tag="lsb",
                                        name="l_sb")[:, :qw]
                        nc.vector.tensor_copy(
                            l_sb, psys[hs][c.HD:c.HD + 1, :])
                        nc.gpsimd.dma_start(
                            l_all[hp * c.HPG + hs:hp * c.HPG + hs + 1, :],
                            l_sb)
                        l_sbs.append(l_sb)
                    for hs in range(c.HPG):
                        pb = hs * c.HD
                        nc.vector.tensor_copy(
                            yu_q[pb:pb + c.HD, hp, :], psys[hs][0:c.HD, :])
                    # hp-boundary fillers: QKV chains of LATER tiles, spread
                    # into this (scalar-bound) region. All writes still
                    # precede their readers in emission order.
                    if fillers:
                        for th in fillers[hp]:
                            th()
                return yu_q, l_all

            def norm(yu_q, l_all, qw):
                # batched softmax normalization for all 8 heads; 1/l only
                # needs bf16 accuracy downstream, so the ~18-bit single-op
                # DVE approximation replaces the slow exact reciprocal
                r32 = lrp.tile([c.H_LOC, c.QT], F32, tag="r32",
                               name="r32")[:, :qw]
                nc.vector.reciprocal_approx_fast(r32, l_all)
                r_all = lrp.tile([c.H_LOC, c.QT], BF16, tag="rall",
                                 name="r_all")[:, :qw]
                nc.vector.tensor_copy(r_all, r32)
                yt_q = ytp.tile([P, c.NHP, c.QT], BF16, name="yt_q")[:, :, :qw]
                for hp in range(c.NHP):
                    psr = ps_mm.tile([P, max(c.TT, c.DH)], F32,
                                     tag="mm", name="psr")[:, :qw]
                    nc.tensor.matmul(
                        psr, esel_sb[:, ts(hp, P)], r_all,
                        start=True, stop=True,
                    )
                    nc.vector.tensor_tensor(
                        yt_q[:, hp, :], yu_q[:, hp, :], psr, ALU.mult,
                    )
                return yt_q

            def ship(q0, qw, yt_q, pieces=1):
                # ship y^T; pairwise AllGather along the dims axis. The
                # final step ships in two half-width pieces: small
                # AllGathers have much lower latency and the first piece's
                # proj overlaps the second piece's gather.
                shipped = []
                pw = qw // pieces
                for pc in range(pieces):
                    y_loc = drp.tile([c.DH, pw], BF16,
                                     tag=f"yloc{pw}", name="y_loc")
                    ylr = y_loc.rearrange("(hp p) t -> p hp t", p=P)
                    for hp in range(c.NHP):
                        nc.sync.dma_start(
                            ylr[:, hp, :],
                            yt_q[:, hp, ds(pc * pw, pw)],
                        )
                    if use_cc:
                        y_ag = drp.tile([c.GDH, pw], BF16,
                                        tag=f"ygat{pw}", name="y_ag")
                        nc.gpsimd.collective_compute(
                            "AllGather", ALU.bypass,
                            replica_groups=groups,
                            ins=[y_loc.opt()], outs=[y_ag.opt()],
                        )
                    else:
                        y_ag = y_loc
                    shipped.append((y_ag, pw, q0 + pc * pw))
                return shipped

            def proj(y_ag, hw_, tok0):
                # column-sharded proj on the gathered y => the host only
                # concatenates, no reduction anywhere
                yag_sb = yagp.tile([P, c.GDH // P, c.QT], BF16,
                                   name="yag_sb")[:, :, :hw_]
                yar = y_ag.rearrange("(ch p) t -> p ch t", p=P)
                for ch in range(c.GDH // P):
                    nc.sync.dma_start(yag_sb[:, ch, :], yar[:, ch, :])
                for tb in range(hw_ // P):
                    pso = ps_mm.tile([P, max(c.TT, c.DH)], F32,
                                     tag="mm", name="pso")[:, :c.DH]
                    for c2 in range(c.GDH // P):
                        nc.tensor.matmul(
                            pso,
                            yag_sb[:, c2, ts(tb, P)],
                            wp_sb[:, c2, :],
                            start=(c2 == 0),
                            stop=(not with_bias
                                  and c2 == c.GDH // P - 1),
                        )
                    if with_bias:
                        nc.tensor.matmul(
                            pso, ones_row[0:1, 0:P], bp_row,
                            start=False, stop=True,
                        )
                    osb = osbp.tile([P, c.DH], F32)
                    nc.vector.tensor_copy(osb, pso)
                    nc.gpsimd.dma_start(out[ds(tok0 + tb * P, P), :], osb)

            # ---- software pipeline over query-tile steps ----
            # The last tile is split in two half-width attention passes so
            # its drain/AllGather hides under the second half's compute.
            # Emission order per step: attention -> next QKV tile ->
            # proj(prev) -> norm -> ship. QKV/proj sit between attention
            # and norm in mm-pool allocation order, so their PSUM slots
            # recycle long-released tiles and the Tile list scheduler can
            # back-fill PE idle slots (ScalarE exp waits) with them.
            steps = [(qt * c.QT, c.QT) for qt in range(c.NQT)]
            if c.QT // 2 >= P:
                q0l = steps.pop()[0]
                h = c.QT // 2
                steps += [(q0l, h), (q0l + h, h)]
            balanced = c.NQT == 4 and c.NTT == 4 and len(steps) == 5

            if balanced:
                # Spread QKV tiles 2 and 3 over the scalar-bound attention
                # regions at hp boundaries: steps 1-2 are PE-bound while
                # the late small-step regions idle the PE, so chains of the
                # NEXT tiles migrate late (each chain still lands before
                # the first attention unit that reads it).
                x2, x3 = [None], [None]

                def kq(xh, tt, cb):
                    return lambda: qkv_kq(xh[0], tt, cb)

                def vv(xh, tt, tb):
                    return lambda: qkv_v(xh[0], tt, tb)

                fill = {
                    1: [[kq(x2, 2, 0)], [kq(x2, 2, 1)],
                        [vv(x2, 2, 0), vv(x2, 2, 1)],
                        [vv(x2, 2, 2), vv(x2, 2, 3)]],
                    2: [[kq(x2, 2, 2)], [kq(x2, 2, 3)],
                        [kq(x3, 3, 0)],
                        [vv(x3, 3, 0), vv(x3, 3, 1)]],
                    3: [[kq(x3, 3, 1), vv(x3, 3, 2)],
                        [kq(x3, 3, 2), vv(x3, 3, 3)],
                        [kq(x3, 3, 3)], []],
                }
            else:
                # attention step s uses K/V tiles <= s, so tile s+1 is
                # emitted during step s
                fill = {}
                qkv_after = {s: s + 1 for s in range(c.NQT - 1)}
            qkv_tile(0, xt=x0)
            pending = []
            for s, (q0, qw) in enumerate(steps):
                last = s == len(steps) - 1
                yu_q, l_all = attention(q0, qw, fill.get(s))
                if balanced:
                    # x DMAs ride a shared ring at ~20-30 GB/s effective:
                    # a 1 MB tile takes most of a step to arrive, so issue
                    # each one a full step before its chains run
                    if s == 0:
                        qkv_tile(1)
                        x2[0] = xt_dma(2)
                    if s == 1:
                        x3[0] = xt_dma(3)
                else:
                    nt = qkv_after.get(s)
                    if nt is not None and nt < c.NTT:
                        qkv_tile(nt)
                if s == 0:
                    # wp is first needed by proj during step 1; deferring
                    # its DMA keeps startup bandwidth for x and w_qkv
                    for ch in range(c.GDH // P):
                        nc.gpsimd.dma_start(wp_sb[:, ch, :], wp[:, ch, :])
                for item in pending:
                    proj(*item)
                # norm/ship head the per-step serial chain into the
                # AllGather: boost them above the qkv/proj filler so the
                # collective is issued as soon as the drain lands
                with tc.high_priority():
                    yt_q = norm(yu_q, l_all, qw)
                    pending = ship(q0, qw, yt_q,
                                   pieces=2 if (last and qw // 2 >= P)
                                   else 1)
            for item in pending:
                proj(*item)

    nc.compile()
    return nc


def arrange_x(c: Cfg, xT):
    """[D, T] -> [P, NTT, DCH, TT]: partition-major, contiguous lines."""
    return np.ascontiguousarray(
        xT.reshape(c.DCH, P, c.NTT, c.TT).transpose(1, 2, 0, 3))


def arrange_w(c: Cfg, w):
    """[nch*P, n] -> [P, nch, n]: partition-major, contiguous lines."""
    nch = w.shape[0] // P
    return np.ascontiguousarray(
        w.reshape(nch, P, w.shape[1]).transpose(1, 0, 2))


def shard_inputs(c: Cfg, x, w_qkv, b_qkv, w_proj, b_proj, n_cores=8):
    """Full fp32 inputs -> per-core input maps (host-side marshalling).

    Matmul operands are cast to bf16 on the host; q/k biases stay fp32
    (applied via ScalarE's per-partition bias port on the f32 PSUM)."""
    D, DH = c.D, c.DH
    oc = max(128, (c.T // 128) * c.H_LOC)
    ones = np.ones((128, oc), BF16NP)
    esel = np.zeros((c.H_LOC, c.NHP * 128), BF16NP)
    for h in range(c.H_LOC):
        hp, sub = h // c.HPG, h % c.HPG
        esel[h, hp * 128 + sub * c.HD: hp * 128 + (sub + 1) * c.HD] = 1
    maps = []
    for core in range(n_cores):
        b, hh = core // c.n_groups, core % c.n_groups
        sl = slice(hh * DH, (hh + 1) * DH)
        maps.append({
            "xT": arrange_x(c, np.ascontiguousarray(x[b].T)).astype(BF16NP),
            "wq": arrange_w(
                c, w_qkv[:, 0 * D:1 * D][:, sl]).astype(BF16NP),
            "wk": arrange_w(
                c, w_qkv[:, 1 * D:2 * D][:, sl]).astype(BF16NP),
            "wv": arrange_w(
                c, w_qkv[:, 2 * D:3 * D][:, sl]).astype(BF16NP),
            "bq": np.ascontiguousarray(
                b_qkv[0 * D:1 * D][sl], dtype=np.float32),
            "bk": np.ascontiguousarray(
                b_qkv[1 * D:2 * D][sl], dtype=np.float32),
            "bv": np.ascontiguousarray(
                b_qkv[2 * D:3 * D][sl]).reshape(1, DH).astype(BF16NP),
            "wp": arrange_w(c, w_proj[:, sl]).astype(BF16NP),
            "bp": np.ascontiguousarray(
                b_proj[sl]).reshape(1, DH).astype(BF16NP),
            "onesin": ones,
            "esel": esel,
        })
    return maps


def gather_outputs(c: Cfg, results, n_cores=8):
    B = n_cores // c.n_groups
    out = np.empty((B, c.T, c.GDH), dtype=np.float32)
    for core in range(n_cores):
        b, hh = core // c.n_groups, core % c.n_groups
        out[b][:, hh * c.DH:(hh + 1) * c.DH] = results[core]["out"]
    return out


_NC_CACHE: dict = {}


def kernel(**inputs) -> np.ndarray:
    from concourse.bass_utils import run_bass_kernel_spmd

    c = FULL
    n_cores = 8
    wb = bool(np.any(inputs["b_qkv"]) or np.any(inputs["b_proj"]))
    key = (c, n_cores, wb)
    if key not in _NC_CACHE:
        _NC_CACHE[key] = build_nc(c, n_cores, with_bias=wb)
    nc = _NC_CACHE[key]
    in_maps = shard_inputs(
        c, inputs["x"], inputs["w_qkv"], inputs["b_qkv"],
        inputs["w_proj"], inputs["b_proj"], n_cores,
    )
    res = run_bass_kernel_spmd(
        nc, in_maps, core_ids=list(range(n_cores)),
        trace=bool(int(os.environ.get("KERNEL_TRACE", "0"))),
    )
    kernel.last_results = res
    return gather_outputs(c, res.results, n_cores)


# revision 20
# speedup vs baseline: 1.2993x; 1.0826x over previous
"""Causal self-attention (B=4, T=2048, D=1024, H=16) on 8 trn2 NeuronCores.

Sharding: batch (4-way) x head-half (2-way tensor parallel) => 8 cores,
one uniform SPMD program (per-core differences are pure data: which batch's
x, which half of the QKV columns / proj columns each core receives).

Per core (batch b, head-half hh, 8 local heads), all matmul operands bf16
(fp32 PSUM accumulation):
  1. QKV: q^T/k^T computed in [qkv_col, token] layout (lhsT = W chunk,
     rhs = x^T chunk); v computed in [token, vcol] layout.
  2. Attention per head pair, per 512-wide query tile, streaming 128-wide
     key blocks (block-causal; fully-masked key blocks are skipped):
       S^T[k,q]  = matmul(lhsT=k^T chunk, rhs=q^T tile)      (PSUM f32)
       both heads of a partition group run in disjoint PE row quadrants
       and land in one [128, 2, 512] PSUM tile, so a single ScalarE
       ACTIVATE computes exp for the pair (amortizes the ~175 ns fixed
       ACT cost, the ScalarE stream is the attention-phase floor).
       y_ext^T  += matmul(lhsT=v_ext block, rhs=P^T): v_ext carries a ones
                   column, so row HD of the accumulator is the softmax
                   denominator l -- no extra reduction pass.
  3. The whole kernel is emitted as one software pipeline over query
     tiles s: attention(s) first (highest priority), then norm+AllGather
     ship(s), then proj(s-1), then QKV(s+1). The Tile list scheduler
     back-fills PE idle slots (waiting on exp) with the ready QKV / proj
     matmuls, which keeps the PE HAM clock-gate warm (2.4 GHz) instead
     of oscillating against the ScalarE exp stream.
  4. Pairwise AllGather of y^T (bf16) between the two cores sharing a
     batch; proj is column-sharded => the host only concatenates.
"""

import os
import sys
from dataclasses import dataclass

import ml_dtypes
import numpy as np

sys.path.insert(0, "/opt/trn_rl_repo")

import concourse.mybir as mybir  # noqa: E402
import concourse.tile as tile  # noqa: E402
from concourse import bacc  # noqa: E402
from concourse.bass import ds, ts  # noqa: E402

P = 128
F32 = mybir.dt.float32
BF16 = mybir.dt.bfloat16
AF = mybir.ActivationFunctionType
ALU = mybir.AluOpType
BF16NP = ml_dtypes.bfloat16


@dataclass(frozen=True)
class Cfg:
    T: int = 2048          # sequence length
    D: int = 1024          # model dim (QKV contraction dim)
    H_LOC: int = 8         # heads per core
    HD: int = 64           # head dim
    TT: int = 512          # token tile width in the QKV phase
    QT: int = 512          # query tile width in the attention phase
    n_groups: int = 2      # cores sharing a batch (pairwise AllGather)
    scale: float = 64 ** -0.5

    @property
    def DH(self):          # local head dims (y^T rows contributed per core)
        return self.H_LOC * self.HD

    @property
    def GDH(self):         # proj contraction dim (= model dim)
        return self.n_groups * self.DH

    @property
    def DCH(self):
        return self.D // P

    @property
    def NHP(self):         # 128-partition groups of local head dims
        return self.DH // P

    @property
    def HPG(self):         # heads per 128-partition group
        return P // self.HD

    @property
    def NTT(self):
        return self.T // self.TT

    @property
    def NQT(self):
        return self.T // self.QT

    @property
    def CB(self):          # 128-wide column blocks of the local q/k cols
        return self.DH // P


FULL = Cfg()


def build_nc(c: Cfg, n_cores: int = 8, with_bias: bool = True):
    """Build the (uniform SPMD) Bass program for one core."""
    assert c.T % c.TT == 0 and c.T % c.QT == 0 and c.QT % P == 0
    assert c.D % P == 0 and c.DH % P == 0 and c.TT % P == 0
    use_cc = c.n_groups > 1

    nc = bacc.Bacc(
        "TRN2", target_bir_lowering=False, debug=False, num_devices=n_cores
    )
    # x and the weights arrive pre-arranged from the host (partition-major,
    # contiguous per partition) so every load is a full-line DMA
    xT = nc.dram_tensor("xT", [P, c.NTT, c.DCH, c.TT], BF16,
                        kind="ExternalInput").ap()
    wq = nc.dram_tensor("wq", [P, c.DCH, c.DH], BF16,
                        kind="ExternalInput").ap()
    wk = nc.dram_tensor("wk", [P, c.DCH, c.DH], BF16,
                        kind="ExternalInput").ap()
    wv = nc.dram_tensor("wv", [P, c.DCH, c.DH], BF16,
                        kind="ExternalInput").ap()
    bq = nc.dram_tensor("bq", [c.DH], F32, kind="ExternalInput").ap()
    bk = nc.dram_tensor("bk", [c.DH], F32, kind="ExternalInput").ap()
    bv = nc.dram_tensor("bv", [1, c.DH], BF16, kind="ExternalInput").ap()
    wp = nc.dram_tensor("wp", [P, c.GDH // P, c.DH], BF16,
                        kind="ExternalInput").ap()
    bp = nc.dram_tensor("bp", [1, c.DH], BF16, kind="ExternalInput").ap()
    oc = max(P, (c.T // P) * c.H_LOC)
    onesin = nc.dram_tensor("onesin", [P, oc], BF16, kind="ExternalInput").ap()
    esel = nc.dram_tensor("esel", [c.H_LOC, c.NHP * P], BF16,
                          kind="ExternalInput").ap()
    out = nc.dram_tensor("out", [c.T, c.DH], F32, kind="ExternalOutput").ap()

    groups = [[g * c.n_groups + i for i in range(c.n_groups)]
              for g in range(max(1, n_cores // c.n_groups))]

    with tile.TileContext(nc) as tc:
        with (
            tc.tile_pool(name="const", bufs=1) as cst,
            tc.tile_pool(name="kv", bufs=1) as kv,
            tc.tile_pool(name="wqkv", bufs=1) as wqk,
            tc.tile_pool(name="wproj", bufs=1) as wpp,
            tc.tile_pool(name="xt", bufs=3) as xtp,
            tc.tile_pool(name="pt", bufs=4) as ptp,
            tc.tile_pool(name="yt", bufs=2) as ytp,
            tc.tile_pool(name="yu", bufs=2) as yup,
            tc.tile_pool(name="lr", bufs=2) as lrp,
            tc.tile_pool(name="yag", bufs=2) as yagp,
            tc.tile_pool(name="osb", bufs=2) as osbp,
            tc.tile_pool(name="ps_mm", bufs=2, space="PSUM") as ps_mm,
            tc.tile_pool(name="ps_s", bufs=2, space="PSUM") as ps_s,
            tc.tile_pool(name="ps_y", bufs=2, space="PSUM") as ps_y,
            tc.tile_pool(name="dram", bufs=2, space="DRAM") as drp,
        ):
            # ---- constants ----
            ones_row = cst.tile([1, P], BF16)
            nc.gpsimd.dma_start(ones_row, onesin[0:1, 0:P])
            bq_sb = cst.tile([P, c.CB], F32)
            nc.scalar.dma_start(bq_sb, bq.rearrange("(cb p) -> p cb", p=P))
            bk_sb = cst.tile([P, c.CB], F32)
            nc.scalar.dma_start(bk_sb, bk.rearrange("(cb p) -> p cb", p=P))
            bv_row = cst.tile([1, c.DH], BF16)
            nc.scalar.dma_start(bv_row, bv)
            bp_row = cst.tile([1, c.DH], BF16)
            nc.scalar.dma_start(bp_row, bp)
            esel_sb = cst.tile([c.H_LOC, c.NHP * P], BF16)
            nc.gpsimd.dma_start(esel_sb, esel)
            # pre-load the exp spline table so the ~2.7us ACT_TABLE_LOAD
            # overlaps the QKV prologue instead of the first S^T block
            warm = cst.tile([1, 8], BF16)
            nc.scalar.activation(warm, ones_row[0:1, 0:8], AF.Exp)
            # causal triangle mask (keep j >= k) for the diagonal key block,
            # replicated per head of a partition group so one DVE multiply
            # masks the whole [128, HPG, 128] slice
            mask2 = cst.tile([P, c.HPG, P], BF16)
            nc.vector.memset(mask2, 1.0)
            for i in range(c.HPG):
                nc.gpsimd.affine_select(
                    mask2[:, i, :], mask2[:, i, :],
                    compare_op=ALU.is_ge, fill=0.0, base=0,
                    pattern=[[1, P]], channel_multiplier=-1,
                )

            # ---- persistent K^T / Q^T / V(+ones) ----
            kT = kv.tile([P, c.NHP, c.T], BF16)
            qT = kv.tile([P, c.NHP, c.T], BF16)
            v = kv.tile([P, c.T // P, c.H_LOC, c.HD + 1], BF16)
            nc.vector.memset(v[:, :, :, c.HD:c.HD + 1], 1.0)

            wq_sb = wqk.tile([P, c.DCH, c.DH], BF16)
            wk_sb = wqk.tile([P, c.DCH, c.DH], BF16)
            wv_sb = wqk.tile([P, c.DCH, c.DH], BF16)
            wp_sb = wpp.tile([P, c.GDH // P, c.DH], BF16)
            # per-dc chunks, interleaved: the first K/Q accumulation
            # chains become runnable after ~2 chunks instead of after the
            # whole 4 MB of weights+x (the prologue is DMA-bound)
            x0 = xtp.tile([P, c.DCH, c.TT], BF16, name="x0")
            for dc in range(c.DCH):
                nc.sync.dma_start(x0[:, dc, :], xT[:, 0, dc, :])
                nc.gpsimd.dma_start(wk_sb[:, dc, :], wk[:, dc, :])
                nc.scalar.dma_start(wq_sb[:, dc, :], wq[:, dc, :])
                nc.gpsimd.dma_start(wv_sb[:, dc, :], wv[:, dc, :])

            def qkv_kq(xt, tt, cb):
                for dst, w_sb, b_sb in (
                    (kT, wk_sb, bk_sb),
                    (qT, wq_sb, bq_sb),
                ):
                    pst = ps_mm.tile([P, max(c.TT, c.DH)], F32,
                                     tag="mm", name="pst")[:, :c.TT]
                    for dc in range(c.DCH):
                        nc.tensor.matmul(
                            pst,
                            w_sb[:, dc, ts(cb, P)],
                            xt[:, dc, :],
                            start=(dc == 0),
                            stop=(dc == c.DCH - 1),
                        )
                    nc.vector.tensor_tensor(
                        dst[:, cb, ts(tt, c.TT)], pst,
                        b_sb[:, cb:cb + 1].to_broadcast((P, c.TT)),
                        ALU.add,
                    )

            def qkv_v(xt, tt, tb):
                gtb = tt * (c.TT // P) + tb
                psv = ps_mm.tile([P, max(c.TT, c.DH)], F32,
                                 tag="mm", name="psv")[:, :c.DH]
                for dc in range(c.DCH):
                    nc.tensor.matmul(
                        psv,
                        xt[:, dc, ts(tb, P)],
                        wv_sb[:, dc, :],
                        start=(dc == 0),
                        stop=(not with_bias and dc == c.DCH - 1),
                    )
                if with_bias:
                    nc.tensor.matmul(
                        psv, ones_row[0:1, 0:P], bv_row,
                        start=False, stop=True,
                    )
                nc.vector.tensor_copy(
                    v[:, gtb, :, 0:c.HD],
                    psv.rearrange("p (h d) -> p h d", d=c.HD),
                )

            def xt_dma(tt):
                # one dma_start entry is serviced by a single DMA engine
                # (~19 GB/s): split every big transfer into per-chunk
                # entries so they spread across the ring's 16 engines
                xt = xtp.tile([P, c.DCH, c.TT], BF16, name="xt")
                for dc in range(c.DCH):
                    nc.sync.dma_start(xt[:, dc, :], xT[:, tt, dc, :])
                return xt

            def qkv_tile(tt, xt=None):
                if xt is None:
                    xt = xt_dma(tt)
                # K/Q col-block 0 and all of V first: the first attention
                # partition group of the NEXT query tile becomes runnable
                # as early as possible (matters most for the prologue tile)
                qkv_kq(xt, tt, 0)
                for tb in range(c.TT // P):
                    qkv_v(xt, tt, tb)
                for cb in range(1, c.CB):
                    qkv_kq(xt, tt, cb)

            def attention(q0, qw, fillers=None):
                nkb = (q0 + qw) // P
                yu_q = yup.tile([P, c.NHP, c.QT], F32, name="yu_q")[:, :, :qw]
                l_all = lrp.tile([c.H_LOC, c.QT], F32, tag="lall",
                                  name="l_all")[:, :qw]
                for hp in range(c.NHP):
                    # the HPG heads sharing this partition group run their
                    # S^T matmuls in disjoint PE row quadrants
                    # (tile_position auto-derived from base_partition) into
                    # one 2-bank PSUM tile => a single ACTIVATE per block.
                    psys = [ps_y.tile([c.HD + 1, c.QT], F32, tag="psy",
                                      name=f"psy{hs}")[:, :qw]
                            for hs in range(c.HPG)]
                    for kb in range(nkb):
                        off = max(0, kb * P - q0)
                        diag = kb * P - q0 >= 0
                        w = qw - off
                        pss = ps_s.tile([P, c.HPG, c.QT], F32, tag="pss",
                                        name="pss")[:, :, :qw]
                        for hs in range(c.HPG):
                            pb = hs * c.HD
                            nc.tensor.matmul(
                                pss[:, hs, off:],
                                kT[pb:pb + c.HD, hp, ts(kb, P)],
                                qT[pb:pb + c.HD, hp, ds(q0 + off, w)],
                                start=True, stop=True,
                            )
                        pt = ptp.tile([P, c.HPG, c.QT], BF16, tag="pt",
                                      name="pt")[:, :, :qw]
                        nc.scalar.activation(
                            pt[:, :, off:], pss[:, :, off:],
                            AF.Exp, scale=c.scale)
                        if diag:
                            # triangle mask on the 128 cols at the diagonal
                            nc.vector.tensor_tensor(
                                pt[:, :, off:off + P],
                                pt[:, :, off:off + P],
                                mask2, ALU.mult,
                            )
                        for hs in range(c.HPG):
                            nc.tensor.matmul(
                                psys[hs][:, off:],
                                v[:, kb, hp * c.HPG + hs, :],
                                pt[:, hs, off:],
                                start=(kb == 0),
                                stop=(kb == nkb - 1),
                            )
                    # stage the denominators first (they head the serial
                    # reciprocal -> norm -> ship chain), then y^T
                    l_sbs = []
                    for hs in range(c.HPG):
                        l_sb = lrp.tile([1, c.QT], F32, tag="lsb",
                                        name="l_sb")[:, :qw]
                        nc.vector.tensor_copy(
                            l_sb, psys[hs][c.HD:c.HD + 1, :])
                        nc.gpsimd.dma_start(
                            l_all[hp * c.HPG + hs:hp * c.HPG + hs + 1, :],
                            l_sb)
                        l_sbs.append(l_sb)
                    for hs in range(c.HPG):
                        pb = hs * c.HD
                        nc.vector.tensor_copy(
                            yu_q[pb:pb + c.HD, hp, :], psys[hs][0:c.HD, :])
                    # hp-boundary fillers: QKV chains of LATER tiles, spread
                    # into this (scalar-bound) region. All writes still
                    # precede their readers in emission order.
                    if fillers:
                        for th in fillers[hp]:
                            th()
                return yu_q, l_all

            def norm(yu_q, l_all, qw):
                # batched softmax normalization for all 8 heads; 1/l only
                # needs bf16 accuracy downstream, so the ~18-bit single-op
                # DVE approximation replaces the slow exact reciprocal
                r32 = lrp.tile([c.H_LOC, c.QT], F32, tag="r32",
                               name="r32")[:, :qw]
                nc.vector.reciprocal_approx_fast(r32, l_all)
                r_all = lrp.tile([c.H_LOC, c.QT], BF16, tag="rall",
                                 name="r_all")[:, :qw]
                nc.vector.tensor_copy(r_all, r32)
                yt_q = ytp.tile([P, c.NHP, c.QT], BF16, name="yt_q")[:, :, :qw]
                for hp in range(c.NHP):
                    psr = ps_mm.tile([P, max(c.TT, c.DH)], F32,
                                     tag="mm", name="psr")[:, :qw]
                    nc.tensor.matmul(
                        psr, esel_sb[:, ts(hp, P)], r_all,
                        start=True, stop=True,
                    )
                    nc.vector.tensor_tensor(
                        yt_q[:, hp, :], yu_q[:, hp, :], psr, ALU.mult,
                    )
                return yt_q

            def ship(q0, qw, yt_q, pieces=1):
                # ship y^T; pairwise AllGather along the dims axis. The
                # final step ships in two half-width pieces: small
                # AllGathers have much lower latency and the first piece's
                # proj overlaps the second piece's gather.
                shipped = []
                pw = qw // pieces
                for pc in range(pieces):
                    y_loc = drp.tile([c.DH, pw], BF16,
                                     tag=f"yloc{pw}", name="y_loc")
                    ylr = y_loc.rearrange("(hp p) t -> p hp t", p=P)
                    for hp in range(c.NHP):
                        nc.sync.dma_start(
                            ylr[:, hp, :],
                            yt_q[:, hp, ds(pc * pw, pw)],
                        )
                    if use_cc:
                        y_ag = drp.tile([c.GDH, pw], BF16,
                                        tag=f"ygat{pw}", name="y_ag")
                        nc.gpsimd.collective_compute(
                            "AllGather", ALU.bypass,
                            replica_groups=groups,
                            ins=[y_loc.opt()], outs=[y_ag.opt()],
                        )
                    else:
                        y_ag = y_loc
                    shipped.append((y_ag, pw, q0 + pc * pw))
                return shipped

            def proj(y_ag, hw_, tok0):
                # column-sharded proj on the gathered y => the host only
                # concatenates, no reduction anywhere
                yag_sb = yagp.tile([P, c.GDH // P, c.QT], BF16,
                                   name="yag_sb")[:, :, :hw_]
                yar = y_ag.rearrange("(ch p) t -> p ch t", p=P)
                for ch in range(c.GDH // P):
                    nc.sync.dma_start(yag_sb[:, ch, :], yar[:, ch, :])
                for tb in range(hw_ // P):
                    pso = ps_mm.tile([P, max(c.TT, c.DH)], F32,
                                     tag="mm", name="pso")[:, :c.DH]
                    for c2 in range(c.GDH // P):
                        nc.tensor.matmul(
                            pso,
                            yag_sb[:, c2, ts(tb, P)],
                            wp_sb[:, c2, :],
                            start=(c2 == 0),
                            stop=(not with_bias
                                  and c2 == c.GDH // P - 1),
                        )
                    if with_bias:
                        nc.tensor.matmul(
                            pso, ones_row[0:1, 0:P], bp_row,
                            start=False, stop=True,
                        )
                    osb = osbp.tile([P, c.DH], F32)
                    nc.vector.tensor_copy(osb, pso)
                    nc.gpsimd.dma_start(out[ds(tok0 + tb * P, P), :], osb)

            # ---- software pipeline over query-tile steps ----
            # The last tile is split in two half-width attention passes so
            # its drain/AllGather hides under the second half's compute.
            # Emission order per step: attention -> next QKV tile ->
            # proj(prev) -> norm -> ship. QKV/proj sit between attention
            # and norm in mm-pool allocation order, so their PSUM slots
            # recycle long-released tiles and the Tile list scheduler can
            # back-fill PE idle slots (ScalarE exp waits) with them.
            steps = [(qt * c.QT, c.QT) for qt in range(c.NQT)]
            balanced = c.NQT == 4 and c.NTT == 4

            if balanced:
                # Spread QKV tiles 2 and 3 over the scalar-bound attention
                # regions at hp boundaries: steps 1-2 are PE-bound while
                # the late small-step regions idle the PE, so chains of the
                # NEXT tiles migrate late (each chain still lands before
                # the first attention unit that reads it).
                x2, x3 = [None], [None]

                def kq(xh, tt, cb):
                    return lambda: qkv_kq(xh[0], tt, cb)

                def vv(xh, tt, tb):
                    return lambda: qkv_v(xh[0], tt, tb)

                fill = {
                    1: [[kq(x2, 2, 0)], [kq(x2, 2, 1)],
                        [vv(x2, 2, 0), vv(x2, 2, 1)],
                        [vv(x2, 2, 2), vv(x2, 2, 3)]],
                    2: [[kq(x2, 2, 2)], [kq(x2, 2, 3)],
                        [kq(x3, 3, 0), vv(x3, 3, 0), vv(x3, 3, 1)],
                        [vv(x3, 3, 2), vv(x3, 3, 3)]],
                    3: [[kq(x3, 3, 1)], [kq(x3, 3, 2)],
                        [kq(x3, 3, 3)], []],
                }
            else:
                # attention step s uses K/V tiles <= s, so tile s+1 is
                # emitted during step s
                fill = {}
                qkv_after = {s: s + 1 for s in range(c.NQT - 1)}
            qkv_tile(0, xt=x0)
            pending = []
            for s, (q0, qw) in enumerate(steps):
                last = s == len(steps) - 1
                yu_q, l_all = attention(q0, qw, fill.get(s))
                if balanced:
                    # x DMAs ride a shared ring at ~20-30 GB/s effective:
                    # a 1 MB tile takes most of a step to arrive, so issue
                    # each one a full step before its chains run
                    if s == 0:
                        qkv_tile(1)
                        x2[0] = xt_dma(2)
                    if s == 1:
                        x3[0] = xt_dma(3)
                else:
                    nt = qkv_after.get(s)
                    if nt is not None and nt < c.NTT:
                        qkv_tile(nt)
                if s == 0:
                    # wp is first needed by proj during step 1; deferring
                    # its DMA keeps startup bandwidth for x and w_qkv
                    for ch in range(c.GDH // P):
                        nc.gpsimd.dma_start(wp_sb[:, ch, :], wp[:, ch, :])
                for item in pending:
                    proj(*item)
                # norm/ship head the per-step serial chain into the
                # AllGather: boost them above the qkv/proj filler so the
                # collective is issued as soon as the drain lands
                with tc.high_priority():
                    yt_q = norm(yu_q, l_all, qw)
                    pending = ship(q0, qw, yt_q,
                                   pieces=2 if (last and qw // 2 >= P)
                                   else 1)
            for item in pending:
                proj(*item)

    nc.compile()
    return nc


def arrange_x(c: Cfg, xT):
    """[D, T] -> [P, NTT, DCH, TT]: partition-major, contiguous lines."""
    return np.ascontiguousarray(
        xT.reshape(c.DCH, P, c.NTT, c.TT).transpose(1, 2, 0, 3))


def arrange_w(c: Cfg, w):
    """[nch*P, n] -> [P, nch, n]: partition-major, contiguous lines."""
    nch = w.shape[0] // P
    return np.ascontiguousarray(
        w.reshape(nch, P, w.shape[1]).transpose(1, 0, 2))


def shard_inputs(c: Cfg, x, w_qkv, b_qkv, w_proj, b_proj, n_cores=8):
    """Full fp32 inputs -> per-core input maps (host-side marshalling).

    Matmul operands are cast to bf16 on the host; q/k biases stay fp32
    (applied via ScalarE's per-partition bias port on the f32 PSUM)."""
    D, DH = c.D, c.DH
    oc = max(128, (c.T // 128) * c.H_LOC)
    ones = np.ones((128, oc), BF16NP)
    esel = np.zeros((c.H_LOC, c.NHP * 128), BF16NP)
    for h in range(c.H_LOC):
        hp, sub = h // c.HPG, h % c.HPG
        esel[h, hp * 128 + sub * c.HD: hp * 128 + (sub + 1) * c.HD] = 1
    maps = []
    for core in range(n_cores):
        b, hh = core // c.n_groups, core % c.n_groups
        sl = slice(hh * DH, (hh + 1) * DH)
        maps.append({
            "xT": arrange_x(c, np.ascontiguousarray(x[b].T)).astype(BF16NP),
            "wq": arrange_w(
                c, w_qkv[:, 0 * D:1 * D][:, sl]).astype(BF16NP),
            "wk": arrange_w(
                c, w_qkv[:, 1 * D:2 * D][:, sl]).astype(BF16NP),
            "wv": arrange_w(
                c, w_qkv[:, 2 * D:3 * D][:, sl]).astype(BF16NP),
            "bq": np.ascontiguousarray(
                b_qkv[0 * D:1 * D][sl], dtype=np.float32),
            "bk": np.ascontiguousarray(
                b_qkv[1 * D:2 * D][sl], dtype=np.float32),
            "bv": np.ascontiguousarray(
                b_qkv[2 * D:3 * D][sl]).reshape(1, DH).astype(BF16NP),
            "wp": arrange_w(c, w_proj[:, sl]).astype(BF16NP),
            "bp": np.ascontiguousarray(
                b_proj[sl]).reshape(1, DH).astype(BF16NP),
            "onesin": ones,
            "esel": esel,
        })
    return maps


def gather_outputs(c: Cfg, results, n_cores=8):
    B = n_cores // c.n_groups
    out = np.empty((B, c.T, c.GDH), dtype=np.float32)
    for core in range(n_cores):
        b, hh = core // c.n_groups, core % c.n_groups
        out[b][:, hh * c.DH:(hh + 1) * c.DH] = results[core]["out"]
    return out


_NC_CACHE: dict = {}


def kernel(**inputs) -> np.ndarray:
    from concourse.bass_utils import run_bass_kernel_spmd

    c = FULL
    n_cores = 8
    wb = bool(np.any(inputs["b_qkv"]) or np.any(inputs["b_proj"]))
    key = (c, n_cores, wb)
    if key not in _NC_CACHE:
        _NC_CACHE[key] = build_nc(c, n_cores, with_bias=wb)
    nc = _NC_CACHE[key]
    in_maps = shard_inputs(
        c, inputs["x"], inputs["w_qkv"], inputs["b_qkv"],
        inputs["w_proj"], inputs["b_proj"], n_cores,
    )
    res = run_bass_kernel_spmd(
        nc, in_maps, core_ids=list(range(n_cores)),
        trace=bool(int(os.environ.get("KERNEL_TRACE", "0"))),
    )
    kernel.last_results = res
    return gather_outputs(c, res.results, n_cores)
